# revision 35
# baseline (speedup 1.0000x reference)
"""Discrete transfer function (2nd-order IIR, Butterworth lowpass) over T=2^23
samples, on 8 TRN2 NeuronCores.

Approach: the filter is stable (poles at radius ~0.577), so the IIR's impulse
response decays below float32 precision within ~100 taps.  The whole recurrence
y = filter(b, a, r), shifted by one sample and scaled by dt, is therefore
exactly (to f32 precision) a 128-tap FIR convolution y[t] = sum_n g[n] r[t-n],
with g derived from (b, a) on the host in float64.

The convolution maps onto the TensorEngine as two banded-Toeplitz matmuls:
with R[k, j] = r[128 j + k] (a [128, J] reshape of the signal),
    Y[c, j] = sum_k W0[k, c] R[k, j] + sum_k W1[k, c] R[k, j-1]
where W0[k, c] = g[c - k] (lower-triangular band) and W1[k, c] = g[c - k + 128]
(strictly upper triangle).  Each core gets a contiguous 2^20-sample chunk plus
a 128-sample halo column from its left neighbor.  The host does the (free)
reshape/transpose so every DMA is 128 partitions x contiguous rows.

Raw bass (not Tile): the pipeline is fully static with explicit per-engine
programs and standalone WAIT semaphore instructions.  (Tile was tried first
but attaches >1 embedded sync-wait to Matmult instructions, which the walrus
codegen rejects; standalone WAIT instructions have no such limit.)

Operands and the output travel as float16 (measured end-to-end rel err vs the
float32 jax reference: 3.2e-4); accumulation is fp32 in PSUM.  Measured on 8
axon-tunneled trn2 cores: ~29 us NEFF exec time per core (all cores run
concurrently), vs a ~23 us full-fp32 memory roofline.
"""

import numpy as np

import concourse.bass as bass
import concourse.mybir as mybir
from concourse.bass_utils import run_bass_kernel_spmd

N_CORES = 8
T = 8388608                 # 2**23, matches the fixed problem size
C = T // N_CORES            # samples per core
J = C // 128                # columns per core in the [128, J] layout
L = 128                     # FIR taps kept (g[n] ~ 0.577^n -> ~1e-30 at n=127)
DT = 1.0                    # module's dt (hardcoded in the source nn.Module)

# Matmul operand dtype.  float32 is exact but runs the PE at 4 cyc/row with a
# serialized per-matmul weight reload; float32r is ~TF32 precision (measured
# 1.6e-4 rel); float16 matches that precision (measured 2.5e-4 rel), halves
# input DMA traffic, and streams at 1 cyc/row with fast weight load.
MM_DT = mybir.dt.float16

_PROGRAM_CACHE: dict = {}

# test.py reads this after calling kernel(..., _trace=True)
LAST_RESULTS = None


def _fir_coeffs(b: np.ndarray, a: np.ndarray) -> np.ndarray:
    """g[n] (length L, float64) with y_out[t] = sum_n g[n] r[t-n].

    g folds together: the IIR impulse response of b/a, the module's
    one-sample output delay, and the dt scale."""
    b = np.asarray(b, np.float64)
    a = np.asarray(a, np.float64)
    a = a / a[0]
    h = np.zeros(L, np.float64)
    for n in range(L):
        acc = b[n] if n < len(b) else 0.0
        for j in range(1, len(a)):
            if n - j >= 0:
                acc -= a[j] * h[n - j]
        h[n] = acc
    g = np.zeros(L, np.float64)
    g[1:] = DT * h[: L - 1]  # y_out[t] = dt * y_iir[t-1], y_out[0] = 0
    return g


def _weight_mats(g: np.ndarray) -> tuple[np.ndarray, np.ndarray]:
    g32 = g.astype(np.float32)
    idx_c = np.arange(128)[None, :]
    idx_k = np.arange(128)[:, None]
    d = idx_c - idx_k  # c - k
    W0 = np.where(d >= 0, g32[np.clip(d, 0, L - 1)], 0.0).astype(np.float32)
    W1 = np.where(d < 0, g32[np.clip(d + 128, 0, L - 1)], 0.0).astype(np.float32)
    return W0, W1


def _build_program(mm_dt):
    """Fully static pipeline; all three DMA queue streams run in parallel.

    Input (fp16, +1 halo col per chunk): chunk sizes [1024,1024,2048,2048,
    2048] cols.  SP HWDGE carries the tiny weight loads first (their
    completion sems gate the PE's first real matmul and SP's ring drains
    FIFO), then c0,c1,c2; ACT HWDGE carries c3; GPSIMD SWDGE carries c4.
    All issued at t=0.

    PE warm-up: while waiting for c0, the PE runs 16 zero matmuls from a
    memset tile so the HAM clock-gate reaches full speed (2.4 GHz) right as
    the real stream begins (cold matmuls run at half clock).

    psum: 8 blocks of BP=1024 cols across 4 [128,1024] PSUM tensors (all 8
    banks); per block 4 matmuls (w0/w1 x 2 x N=512), each 512-col sub-tile
    reading from the single chunk buffer containing its window.  PSUM->SBUF
    drain is split by bank: DVE casts cols [0,512) f32->fp16, ACT casts
    [512,1024), in parallel, into 4 [128,1024] fp16 yout slots.

    Output (fp16; host upcasts): 8 pieces of 1024 cols (256 KiB), piece p
    issued once both half-copies land: GPSIMD takes p0,p2,p4; ACT takes
    p1,p3,p5,p6; SP (idle after its inputs) and ACT split the tail p7 in
    half, each shipping one half-copy the moment it lands.

    All waits are standalone WAIT instructions, and no DMA-completion
    threshold can be met by a mixture of partial completions from different
    transfers (per-chunk input sems; per-slot output sems whose cross-engine
    reuse is ordered by the copy/issue dependency chain)."""
    f32 = mybir.dt.float32
    BP = 1024
    NP = J // BP         # 8 psum blocks / output pieces
    CUTS = [0, 1024, 2048, 4096, 6144, J]
    NCH = len(CUTS) - 1
    N_WARMUP = 16
    # chunk->ring placement chosen so completion order matches the PE's
    # consumption order: c0,c1 lead on SP, c2 on the otherwise-idle ACT ring,
    # c3 on GPSIMD, c4 back on SP where it lands just as blocks 6-7 need it
    IN_ENG = {0: "sp", 1: "sp", 4: "sp", 2: "act", 3: "pool"}
    OUT_ENG = {0: "pool", 2: "pool", 4: "pool", 1: "act", 3: "act", 5: "act", 6: "act", 7: "sp"}
    nc = bass.Bass()
    R = nc.dram_tensor("R", [128, J + 1], mm_dt, kind="ExternalInput")
    W0 = nc.dram_tensor("W0", [128, 128], mm_dt, kind="ExternalInput")
    W1 = nc.dram_tensor("W1", [128, 128], mm_dt, kind="ExternalInput")
    Y = nc.dram_tensor("Y", [128, J], mm_dt, kind="ExternalOutput")

    import contextlib

    with contextlib.ExitStack() as ctx:
        w0t = ctx.enter_context(nc.sbuf_tensor([128, 128], mm_dt))
        w1t = ctx.enter_context(nc.sbuf_tensor([128, 128], mm_dt))
        rin_all = ctx.enter_context(nc.sbuf_tensor([128, J + NCH], mm_dt))
        yout_all = ctx.enter_context(nc.sbuf_tensor([128, 8 * BP], mm_dt))
        dz = ctx.enter_context(nc.sbuf_tensor([128, 512], mm_dt))
        pts = [
            ctx.enter_context(nc.psum_tensor(f"pt{i}", [128, BP], f32))
            for i in range(4)
        ]
        s_w = ctx.enter_context(nc.semaphore("s_w"))
        s_z = ctx.enter_context(nc.semaphore("s_z"))
        s_in = [ctx.enter_context(nc.semaphore(f"s_in{i}")) for i in range(NCH)]
        s_yo = [ctx.enter_context(nc.semaphore(f"s_yo{i}")) for i in range(NP)]
        s_pe = ctx.enter_context(nc.semaphore("s_pe"))
        s_cpv = ctx.enter_context(nc.semaphore("s_cpv"))
        s_cpa = ctx.enter_context(nc.semaphore("s_cpa"))
        s_yo7a = ctx.enter_context(nc.semaphore("s_yo7a"))
        s_yo7b = ctx.enter_context(nc.semaphore("s_yo7b"))
        block = ctx.enter_context(nc.Block())

        # rin[c] holds R cols [CUTS[c], CUTS[c+1]] inclusive (left halo col)
        rin_off = [CUTS[c] + c for c in range(NCH)]
        rin = [
            rin_all[:, rin_off[c] : rin_off[c] + (CUTS[c + 1] - CUTS[c]) + 1]
            for c in range(NCH)
        ]
        yout = [yout_all[:, i * BP : (i + 1) * BP] for i in range(NP)]

        def chunk_of_sub(k):
            # 512-col sub-tile k reads buffer cols [512k, 512(k+1)] inclusive
            for c in range(NCH):
                if CUTS[c] <= 512 * k and 512 * (k + 1) <= CUTS[c + 1]:
                    return c
            raise AssertionError(k)

        def in_chunk(engine, c):
            engine.dma_start(
                out=rin[c], in_=R[:, CUTS[c] : CUTS[c + 1] + 1]
            ).then_inc(s_in[c], 16)

        def out_piece(engine, p, own_copy=False):
            engine.wait_ge(s_cpv, p + 1)
            if not own_copy:
                engine.wait_ge(s_cpa, p + 1)
            engine.dma_start(
                out=Y[:, p * BP : (p + 1) * BP], in_=yout[p]
            ).then_inc(s_yo[p], 16)

        def final_waits(engine, pieces):
            for p in sorted(pieces):
                engine.wait_ge(s_yo[p], 16)

        def eng_pieces(name):
            return sorted(p for p, e in OUT_ENG.items() if e == name)

        def eng_chunks(name):
            return sorted(c for c, e in IN_ENG.items() if e == name)

        @block.sync
        def _(sync):
            sync.dma_start(out=w0t[:, :], in_=W0[:, :]).then_inc(s_w, 16)
            sync.dma_start(out=w1t[:, :], in_=W1[:, :]).then_inc(s_w, 16)
            for c in eng_chunks("sp"):
                in_chunk(sync, c)
            # tail piece p7 is split: SP carries the DVE half only
            sync.wait_ge(s_cpv, NP)
            sync.dma_start(
                out=Y[:, (NP - 1) * BP : (NP - 1) * BP + 512],
                in_=yout[NP - 1][:, :512],
            ).then_inc(s_yo7a, 16)
            sync.wait_ge(s_yo7a, 16)

        @block.tensor
        def _(tensor):
            # HAM warm-up on zeros while the first chunk is in flight
            tensor.wait_ge(s_z, 1)
            for _ in range(N_WARMUP):
                nc.tensor.matmul(
                    pts[0][:, :512], dz[:, :128], dz[:, :], start=True, stop=True
                )
            tensor.wait_ge(s_w, 32)
            waited = set()
            for p in range(NP):
                subs = [2 * p, 2 * p + 1]
                for k in subs:
                    ch = chunk_of_sub(k)
                    if ch not in waited:
                        tensor.wait_ge(s_in[ch], 16)
                        waited.add(ch)
                if p >= 4:
                    # psum slot p%4 drained by both copy engines
                    tensor.wait_ge(s_cpv, p - 3)
                    tensor.wait_ge(s_cpa, p - 3)
                pt = pts[p % 4]
                for s, k in enumerate(subs):
                    ch = chunk_of_sub(k)
                    off = 512 * k - CUTS[ch]
                    nc.tensor.matmul(
                        pt[:, s * 512 : (s + 1) * 512],
                        w0t[:, :],
                        rin[ch][:, off + 1 : off + 513],
                        start=True,
                        stop=False,
                    )
                for s, k in enumerate(subs):
                    ch = chunk_of_sub(k)
                    off = 512 * k - CUTS[ch]
                    mm = nc.tensor.matmul(
                        pt[:, s * 512 : (s + 1) * 512],
                        w1t[:, :],
                        rin[ch][:, off : off + 512],
                        start=False,
                        stop=True,
                    )
                mm.then_inc(s_pe, 1)

        @block.vector
        def _(vector):
            nc.vector.memset(dz[:, :], 0).then_inc(s_z, 1)
            for p in range(NP):
                vector.wait_ge(s_pe, p + 1)
                nc.vector.tensor_copy(
                    out=yout[p][:, :512],
                    in_=pts[p % 4][:, :512],
                ).then_inc(s_cpv, 1)

        @block.scalar
        def _(scalar):
            for c in eng_chunks("act"):
                in_chunk(scalar, c)
            act_out = set(eng_pieces("act"))
            for p in range(NP):
                scalar.wait_ge(s_pe, p + 1)
                nc.scalar.copy(
                    out=yout[p][:, 512:],
                    in_=pts[p % 4][:, 512:],
                ).then_inc(s_cpa, 1)
                if p in act_out:
                    out_piece(scalar, p, own_copy=True)
                if p == NP - 1:
                    # tail piece is split: ACT ships its own copied half,
                    # in parallel with SP's half
                    scalar.dma_start(
                        out=Y[:, p * BP + 512 : (p + 1) * BP],
                        in_=yout[p][:, 512:],
                    ).then_inc(s_yo7b, 16)
            final_waits(scalar, eng_pieces("act"))
            scalar.wait_ge(s_yo7b, 16)

        @block.gpsimd
        def _(gpsimd):
            for c in eng_chunks("pool"):
                in_chunk(gpsimd, c)
            for p in eng_pieces("pool"):
                out_piece(gpsimd, p)
            final_waits(gpsimd, eng_pieces("pool"))

    return nc


def _get_program(mm_dt):
    key = str(mm_dt)
    if key not in _PROGRAM_CACHE:
        _PROGRAM_CACHE[key] = _build_program(mm_dt)
    return _PROGRAM_CACHE[key]


def _device_run(in_maps, _trace: bool = False, **_trace_kwargs):
    """Execute the SPMD program; returns the per-core Y arrays."""
    global LAST_RESULTS
    nc = _get_program(MM_DT)
    res = run_bass_kernel_spmd(
        nc, in_maps, core_ids=list(range(N_CORES)), trace=_trace, **_trace_kwargs
    )
    LAST_RESULTS = res
    return [res.results[m]["Y"] for m in range(N_CORES)]


def _device_run_subprocess(in_maps):
    """Run the kernel in a fresh python process.

    The axon-tunneled runtime occasionally reports a transient
    NRT_EXEC_UNIT_UNRECOVERABLE on an execute; the wedged state does not
    clear within the process, but a fresh process (fresh NRT session)
    recovers reliably."""
    import os
    import subprocess
    import sys
    import tempfile

    tmp = tempfile.mkdtemp()
    in_path = os.path.join(tmp, "in.npz")
    out_path = os.path.join(tmp, "out.npz")
    flat = {}
    for m, im in enumerate(in_maps):
        for k, v in im.items():
            flat[f"{k}_{m}"] = v
    np.savez(in_path, **flat)
    subprocess.run(
        [sys.executable, os.path.abspath(__file__), "--device-run", in_path, out_path],
        check=True,
    )
    with np.load(out_path) as z:
        return [z[f"Y_{m}"] for m in range(N_CORES)]


def kernel(r, b, a, _trace: bool = False, **_trace_kwargs):
    r = np.ascontiguousarray(np.asarray(r, dtype=np.float32))
    assert r.shape == (T,), r.shape

    g = _fir_coeffs(b, a)
    W0, W1 = _weight_mats(g)

    np_dt = mybir.dt.np(MM_DT)  # float16 for 2-byte operands, else float32
    W0 = W0.astype(np_dt)
    W1 = W1.astype(np_dt)

    in_maps = []
    for m in range(N_CORES):
        halo = (
            np.zeros(128, np.float32) if m == 0 else r[m * C - 128 : m * C]
        )
        rbuf = np.concatenate([halo, r[m * C : (m + 1) * C]]).astype(np_dt)
        R = np.ascontiguousarray(rbuf.reshape(J + 1, 128).T)
        in_maps.append({"R": R, "W0": W0, "W1": W1})

    try:
        ys = _device_run(in_maps, _trace=_trace, **_trace_kwargs)
    except Exception:
        ys = _device_run_subprocess(in_maps)

    y = np.concatenate(
        [
            np.ascontiguousarray(ys[m].astype(np.float32, copy=False).T).reshape(-1)
            for m in range(N_CORES)
        ]
    )
    return y


if __name__ == "__main__":
    import sys as _sys

    if len(_sys.argv) == 4 and _sys.argv[1] == "--device-run":
        with np.load(_sys.argv[2]) as _z:
            _in_maps = [
                {k: _z[f"{k}_{m}"] for k in ("R", "W0", "W1")}
                for m in range(N_CORES)
            ]
        _ys = _device_run(_in_maps)
        np.savez(_sys.argv[3], **{f"Y_{m}": _ys[m] for m in range(N_CORES)})


# revision 36
# speedup vs baseline: 1.0531x; 1.0531x over previous
"""Discrete transfer function (2nd-order IIR, Butterworth lowpass) over T=2^23
samples, on 8 TRN2 NeuronCores.

Approach: the filter is stable (poles at radius ~0.577), so the IIR's impulse
response decays below float32 precision within ~100 taps.  The whole recurrence
y = filter(b, a, r), shifted by one sample and scaled by dt, is therefore
exactly (to f32 precision) a 128-tap FIR convolution y[t] = sum_n g[n] r[t-n],
with g derived from (b, a) on the host in float64.

The convolution maps onto the TensorEngine as two banded-Toeplitz matmuls:
with R[k, j] = r[128 j + k] (a [128, J] reshape of the signal),
    Y[c, j] = sum_k W0[k, c] R[k, j] + sum_k W1[k, c] R[k, j-1]
where W0[k, c] = g[c - k] (lower-triangular band) and W1[k, c] = g[c - k + 128]
(strictly upper triangle).  Each core gets a contiguous 2^20-sample chunk plus
a 128-sample halo column from its left neighbor.  The host does the (free)
reshape/transpose so every DMA is 128 partitions x contiguous rows.

Raw bass (not Tile): the pipeline is fully static with explicit per-engine
programs and standalone WAIT semaphore instructions.  (Tile was tried first
but attaches >1 embedded sync-wait to Matmult instructions, which the walrus
codegen rejects; standalone WAIT instructions have no such limit.)

Operands and the output travel as float16 (measured end-to-end rel err vs the
float32 jax reference: 3.2e-4); accumulation is fp32 in PSUM.  Measured on 8
axon-tunneled trn2 cores: ~29 us NEFF exec time per core (all cores run
concurrently), vs a ~23 us full-fp32 memory roofline.
"""

import numpy as np

import concourse.bass as bass
import concourse.mybir as mybir
from concourse.bass_utils import run_bass_kernel_spmd

N_CORES = 8
T = 8388608                 # 2**23, matches the fixed problem size
C = T // N_CORES            # samples per core
J = C // 128                # columns per core in the [128, J] layout
L = 128                     # FIR taps kept (g[n] ~ 0.577^n -> ~1e-30 at n=127)
DT = 1.0                    # module's dt (hardcoded in the source nn.Module)

# Matmul operand dtype.  float32 is exact but runs the PE at 4 cyc/row with a
# serialized per-matmul weight reload; float32r is ~TF32 precision (measured
# 1.6e-4 rel); float16 matches that precision (measured 2.5e-4 rel), halves
# input DMA traffic, and streams at 1 cyc/row with fast weight load.
MM_DT = mybir.dt.float16

_PROGRAM_CACHE: dict = {}

# test.py reads this after calling kernel(..., _trace=True)
LAST_RESULTS = None


def _fir_coeffs(b: np.ndarray, a: np.ndarray) -> np.ndarray:
    """g[n] (length L, float64) with y_out[t] = sum_n g[n] r[t-n].

    g folds together: the IIR impulse response of b/a, the module's
    one-sample output delay, and the dt scale."""
    b = np.asarray(b, np.float64)
    a = np.asarray(a, np.float64)
    a = a / a[0]
    h = np.zeros(L, np.float64)
    for n in range(L):
        acc = b[n] if n < len(b) else 0.0
        for j in range(1, len(a)):
            if n - j >= 0:
                acc -= a[j] * h[n - j]
        h[n] = acc
    g = np.zeros(L, np.float64)
    g[1:] = DT * h[: L - 1]  # y_out[t] = dt * y_iir[t-1], y_out[0] = 0
    return g


def _weight_mats(g: np.ndarray) -> tuple[np.ndarray, np.ndarray]:
    g32 = g.astype(np.float32)
    idx_c = np.arange(128)[None, :]
    idx_k = np.arange(128)[:, None]
    d = idx_c - idx_k  # c - k
    W0 = np.where(d >= 0, g32[np.clip(d, 0, L - 1)], 0.0).astype(np.float32)
    W1 = np.where(d < 0, g32[np.clip(d + 128, 0, L - 1)], 0.0).astype(np.float32)
    return W0, W1


def _build_program(mm_dt):
    """Fully static pipeline; all three DMA queue streams run in parallel.

    Input (fp16, +1 halo col per chunk): chunk sizes [1024,1024,2048,2048,
    2048] cols.  SP HWDGE carries the tiny weight loads first (their
    completion sems gate the PE's first real matmul and SP's ring drains
    FIFO), then c0,c1,c2; ACT HWDGE carries c3; GPSIMD SWDGE carries c4.
    All issued at t=0.

    PE warm-up: while waiting for c0, the PE runs 16 zero matmuls from a
    memset tile so the HAM clock-gate reaches full speed (2.4 GHz) right as
    the real stream begins (cold matmuls run at half clock).

    psum: 8 blocks of BP=1024 cols across 4 [128,1024] PSUM tensors (all 8
    banks); per block 4 matmuls (w0/w1 x 2 x N=512), each 512-col sub-tile
    reading from the single chunk buffer containing its window.  PSUM->SBUF
    drain is split by bank: DVE casts cols [0,512) f32->fp16, ACT casts
    [512,1024), in parallel, into 4 [128,1024] fp16 yout slots.

    Output (fp16; host upcasts): 8 pieces of 1024 cols (256 KiB), piece p
    issued once both half-copies land: GPSIMD takes p0,p2,p4; ACT takes
    p1,p3,p5,p6; SP (idle after its inputs) and ACT split the tail p7 in
    half, each shipping one half-copy the moment it lands.

    All waits are standalone WAIT instructions, and no DMA-completion
    threshold can be met by a mixture of partial completions from different
    transfers (per-chunk input sems; per-slot output sems whose cross-engine
    reuse is ordered by the copy/issue dependency chain)."""
    f32 = mybir.dt.float32
    BP = 1024
    NP = J // BP         # 8 psum blocks / output pieces
    CUTS = [0, 1024, 2048, 4096, 6144, J]
    NCH = len(CUTS) - 1
    N_WARMUP = 16
    # chunk->ring placement chosen so completion order matches the PE's
    # consumption order: only the lead chunks c0,c1 ride SP (short ring ->
    # earliest completion sems), c2 then c4 on ACT, c3 on GPSIMD
    IN_ENG = {0: "sp", 1: "sp", 2: "act", 4: "act", 3: "pool"}
    OUT_ENG = {0: "pool", 2: "pool", 4: "pool", 1: "act", 3: "act", 5: "act", 6: "act", 7: "sp"}
    nc = bass.Bass()
    R = nc.dram_tensor("R", [128, J + 1], mm_dt, kind="ExternalInput")
    W0 = nc.dram_tensor("W0", [128, 128], mm_dt, kind="ExternalInput")
    W1 = nc.dram_tensor("W1", [128, 128], mm_dt, kind="ExternalInput")
    Y = nc.dram_tensor("Y", [128, J], mm_dt, kind="ExternalOutput")

    import contextlib

    with contextlib.ExitStack() as ctx:
        w0t = ctx.enter_context(nc.sbuf_tensor([128, 128], mm_dt))
        w1t = ctx.enter_context(nc.sbuf_tensor([128, 128], mm_dt))
        rin_all = ctx.enter_context(nc.sbuf_tensor([128, J + NCH], mm_dt))
        yout_all = ctx.enter_context(nc.sbuf_tensor([128, 8 * BP], mm_dt))
        dz = ctx.enter_context(nc.sbuf_tensor([128, 512], mm_dt))
        pts = [
            ctx.enter_context(nc.psum_tensor(f"pt{i}", [128, BP], f32))
            for i in range(4)
        ]
        s_w = ctx.enter_context(nc.semaphore("s_w"))
        s_z = ctx.enter_context(nc.semaphore("s_z"))
        s_in = [ctx.enter_context(nc.semaphore(f"s_in{i}")) for i in range(NCH)]
        s_yo = [ctx.enter_context(nc.semaphore(f"s_yo{i}")) for i in range(NP)]
        s_pe = ctx.enter_context(nc.semaphore("s_pe"))
        s_cpv = ctx.enter_context(nc.semaphore("s_cpv"))
        s_cpa = ctx.enter_context(nc.semaphore("s_cpa"))
        s_yo7a = ctx.enter_context(nc.semaphore("s_yo7a"))
        s_yo7b = ctx.enter_context(nc.semaphore("s_yo7b"))
        block = ctx.enter_context(nc.Block())

        # rin[c] holds R cols [CUTS[c], CUTS[c+1]] inclusive (left halo col)
        rin_off = [CUTS[c] + c for c in range(NCH)]
        rin = [
            rin_all[:, rin_off[c] : rin_off[c] + (CUTS[c + 1] - CUTS[c]) + 1]
            for c in range(NCH)
        ]
        yout = [yout_all[:, i * BP : (i + 1) * BP] for i in range(NP)]

        def chunk_of_sub(k):
            # 512-col sub-tile k reads buffer cols [512k, 512(k+1)] inclusive
            for c in range(NCH):
                if CUTS[c] <= 512 * k and 512 * (k + 1) <= CUTS[c + 1]:
                    return c
            raise AssertionError(k)

        def in_chunk(engine, c):
            engine.dma_start(
                out=rin[c], in_=R[:, CUTS[c] : CUTS[c + 1] + 1]
            ).then_inc(s_in[c], 16)

        def out_piece(engine, p, own_copy=False):
            engine.wait_ge(s_cpv, p + 1)
            if not own_copy:
                engine.wait_ge(s_cpa, p + 1)
            engine.dma_start(
                out=Y[:, p * BP : (p + 1) * BP], in_=yout[p]
            ).then_inc(s_yo[p], 16)

        def final_waits(engine, pieces):
            for p in sorted(pieces):
                engine.wait_ge(s_yo[p], 16)

        def eng_pieces(name):
            return sorted(p for p, e in OUT_ENG.items() if e == name)

        def eng_chunks(name):
            return sorted(c for c, e in IN_ENG.items() if e == name)

        @block.sync
        def _(sync):
            sync.dma_start(out=w0t[:, :], in_=W0[:, :]).then_inc(s_w, 16)
            sync.dma_start(out=w1t[:, :], in_=W1[:, :]).then_inc(s_w, 16)
            for c in eng_chunks("sp"):
                in_chunk(sync, c)
            # tail piece p7 is split: SP carries the DVE half only
            sync.wait_ge(s_cpv, NP)
            sync.dma_start(
                out=Y[:, (NP - 1) * BP : (NP - 1) * BP + 512],
                in_=yout[NP - 1][:, :512],
            ).then_inc(s_yo7a, 16)
            sync.wait_ge(s_yo7a, 16)

        @block.tensor
        def _(tensor):
            # HAM warm-up on zeros while the first chunk is in flight
            tensor.wait_ge(s_z, 1)
            for _ in range(N_WARMUP):
                nc.tensor.matmul(
                    pts[0][:, :512], dz[:, :128], dz[:, :], start=True, stop=True
                )
            tensor.wait_ge(s_w, 32)
            waited = set()
            for p in range(NP):
                subs = [2 * p, 2 * p + 1]
                for k in subs:
                    ch = chunk_of_sub(k)
                    if ch not in waited:
                        tensor.wait_ge(s_in[ch], 16)
                        waited.add(ch)
                if p >= 4:
                    # psum slot p%4 drained by both copy engines
                    tensor.wait_ge(s_cpv, p - 3)
                    tensor.wait_ge(s_cpa, p - 3)
                pt = pts[p % 4]
                for s, k in enumerate(subs):
                    ch = chunk_of_sub(k)
                    off = 512 * k - CUTS[ch]
                    nc.tensor.matmul(
                        pt[:, s * 512 : (s + 1) * 512],
                        w0t[:, :],
                        rin[ch][:, off + 1 : off + 513],
                        start=True,
                        stop=False,
                    )
                for s, k in enumerate(subs):
                    ch = chunk_of_sub(k)
                    off = 512 * k - CUTS[ch]
                    mm = nc.tensor.matmul(
                        pt[:, s * 512 : (s + 1) * 512],
                        w1t[:, :],
                        rin[ch][:, off : off + 512],
                        start=False,
                        stop=True,
                    )
                mm.then_inc(s_pe, 1)

        @block.vector
        def _(vector):
            nc.vector.memset(dz[:, :], 0).then_inc(s_z, 1)
            for p in range(NP):
                vector.wait_ge(s_pe, p + 1)
                nc.vector.tensor_copy(
                    out=yout[p][:, :512],
                    in_=pts[p % 4][:, :512],
                ).then_inc(s_cpv, 1)

        @block.scalar
        def _(scalar):
            for c in eng_chunks("act"):
                in_chunk(scalar, c)
            act_out = set(eng_pieces("act"))
            for p in range(NP):
                scalar.wait_ge(s_pe, p + 1)
                nc.scalar.copy(
                    out=yout[p][:, 512:],
                    in_=pts[p % 4][:, 512:],
                ).then_inc(s_cpa, 1)
                if p in act_out:
                    out_piece(scalar, p, own_copy=True)
                if p == NP - 1:
                    # tail piece is split: ACT ships its own copied half,
                    # in parallel with SP's half
                    scalar.dma_start(
                        out=Y[:, p * BP + 512 : (p + 1) * BP],
                        in_=yout[p][:, 512:],
                    ).then_inc(s_yo7b, 16)
            final_waits(scalar, eng_pieces("act"))
            scalar.wait_ge(s_yo7b, 16)

        @block.gpsimd
        def _(gpsimd):
            for c in eng_chunks("pool"):
                in_chunk(gpsimd, c)
            for p in eng_pieces("pool"):
                out_piece(gpsimd, p)
            final_waits(gpsimd, eng_pieces("pool"))

    return nc


def _get_program(mm_dt):
    key = str(mm_dt)
    if key not in _PROGRAM_CACHE:
        _PROGRAM_CACHE[key] = _build_program(mm_dt)
    return _PROGRAM_CACHE[key]


def _device_run(in_maps, _trace: bool = False, **_trace_kwargs):
    """Execute the SPMD program; returns the per-core Y arrays."""
    global LAST_RESULTS
    nc = _get_program(MM_DT)
    res = run_bass_kernel_spmd(
        nc, in_maps, core_ids=list(range(N_CORES)), trace=_trace, **_trace_kwargs
    )
    LAST_RESULTS = res
    return [res.results[m]["Y"] for m in range(N_CORES)]


def _device_run_subprocess(in_maps):
    """Run the kernel in a fresh python process.

    The axon-tunneled runtime occasionally reports a transient
    NRT_EXEC_UNIT_UNRECOVERABLE on an execute; the wedged state does not
    clear within the process, but a fresh process (fresh NRT session)
    recovers reliably."""
    import os
    import subprocess
    import sys
    import tempfile

    tmp = tempfile.mkdtemp()
    in_path = os.path.join(tmp, "in.npz")
    out_path = os.path.join(tmp, "out.npz")
    flat = {}
    for m, im in enumerate(in_maps):
        for k, v in im.items():
            flat[f"{k}_{m}"] = v
    np.savez(in_path, **flat)
    subprocess.run(
        [sys.executable, os.path.abspath(__file__), "--device-run", in_path, out_path],
        check=True,
    )
    with np.load(out_path) as z:
        return [z[f"Y_{m}"] for m in range(N_CORES)]


def kernel(r, b, a, _trace: bool = False, **_trace_kwargs):
    r = np.ascontiguousarray(np.asarray(r, dtype=np.float32))
    assert r.shape == (T,), r.shape

    g = _fir_coeffs(b, a)
    W0, W1 = _weight_mats(g)

    np_dt = mybir.dt.np(MM_DT)  # float16 for 2-byte operands, else float32
    W0 = W0.astype(np_dt)
    W1 = W1.astype(np_dt)

    in_maps = []
    for m in range(N_CORES):
        halo = (
            np.zeros(128, np.float32) if m == 0 else r[m * C - 128 : m * C]
        )
        rbuf = np.concatenate([halo, r[m * C : (m + 1) * C]]).astype(np_dt)
        R = np.ascontiguousarray(rbuf.reshape(J + 1, 128).T)
        in_maps.append({"R": R, "W0": W0, "W1": W1})

    try:
        ys = _device_run(in_maps, _trace=_trace, **_trace_kwargs)
    except Exception:
        ys = _device_run_subprocess(in_maps)

    y = np.concatenate(
        [
            np.ascontiguousarray(ys[m].astype(np.float32, copy=False).T).reshape(-1)
            for m in range(N_CORES)
        ]
    )
    return y


if __name__ == "__main__":
    import sys as _sys

    if len(_sys.argv) == 4 and _sys.argv[1] == "--device-run":
        with np.load(_sys.argv[2]) as _z:
            _in_maps = [
                {k: _z[f"{k}_{m}"] for k in ("R", "W0", "W1")}
                for m in range(N_CORES)
            ]
        _ys = _device_run(_in_maps)
        np.savez(_sys.argv[3], **{f"Y_{m}": _ys[m] for m in range(N_CORES)})


# revision 37
# speedup vs baseline: 1.0997x; 1.0443x over previous
"""Discrete transfer function (2nd-order IIR, Butterworth lowpass) over T=2^23
samples, on 8 TRN2 NeuronCores.

Approach: the filter is stable (poles at radius ~0.577), so the IIR's impulse
response decays below float32 precision within ~100 taps.  The whole recurrence
y = filter(b, a, r), shifted by one sample and scaled by dt, is therefore
exactly (to f32 precision) a 128-tap FIR convolution y[t] = sum_n g[n] r[t-n],
with g derived from (b, a) on the host in float64.

The convolution maps onto the TensorEngine as two banded-Toeplitz matmuls:
with R[k, j] = r[128 j + k] (a [128, J] reshape of the signal),
    Y[c, j] = sum_k W0[k, c] R[k, j] + sum_k W1[k, c] R[k, j-1]
where W0[k, c] = g[c - k] (lower-triangular band) and W1[k, c] = g[c - k + 128]
(strictly upper triangle).  Each core gets a contiguous 2^20-sample chunk plus
a 128-sample halo column from its left neighbor.  The host does the (free)
reshape/transpose so every DMA is 128 partitions x contiguous rows.

Raw bass (not Tile): the pipeline is fully static with explicit per-engine
programs and standalone WAIT semaphore instructions.  (Tile was tried first
but attaches >1 embedded sync-wait to Matmult instructions, which the walrus
codegen rejects; standalone WAIT instructions have no such limit.)

Operands and the output travel as float16 (measured end-to-end rel err vs the
float32 jax reference: 3.2e-4); accumulation is fp32 in PSUM.  Measured on 8
axon-tunneled trn2 cores: ~29 us NEFF exec time per core (all cores run
concurrently), vs a ~23 us full-fp32 memory roofline.
"""

import numpy as np

import concourse.bass as bass
import concourse.mybir as mybir
from concourse.bass_utils import run_bass_kernel_spmd

N_CORES = 8
T = 8388608                 # 2**23, matches the fixed problem size
C = T // N_CORES            # samples per core
J = C // 128                # columns per core in the [128, J] layout
L = 128                     # FIR taps kept (g[n] ~ 0.577^n -> ~1e-30 at n=127)
DT = 1.0                    # module's dt (hardcoded in the source nn.Module)

# Matmul operand dtype.  float32 is exact but runs the PE at 4 cyc/row with a
# serialized per-matmul weight reload; float32r is ~TF32 precision (measured
# 1.6e-4 rel); float16 matches that precision (measured 2.5e-4 rel), halves
# input DMA traffic, and streams at 1 cyc/row with fast weight load.
MM_DT = mybir.dt.float16

_PROGRAM_CACHE: dict = {}

# test.py reads this after calling kernel(..., _trace=True)
LAST_RESULTS = None


def _fir_coeffs(b: np.ndarray, a: np.ndarray) -> np.ndarray:
    """g[n] (length L, float64) with y_out[t] = sum_n g[n] r[t-n].

    g folds together: the IIR impulse response of b/a, the module's
    one-sample output delay, and the dt scale."""
    b = np.asarray(b, np.float64)
    a = np.asarray(a, np.float64)
    a = a / a[0]
    h = np.zeros(L, np.float64)
    for n in range(L):
        acc = b[n] if n < len(b) else 0.0
        for j in range(1, len(a)):
            if n - j >= 0:
                acc -= a[j] * h[n - j]
        h[n] = acc
    g = np.zeros(L, np.float64)
    g[1:] = DT * h[: L - 1]  # y_out[t] = dt * y_iir[t-1], y_out[0] = 0
    return g


def _weight_mats(g: np.ndarray) -> tuple[np.ndarray, np.ndarray]:
    g32 = g.astype(np.float32)
    idx_c = np.arange(128)[None, :]
    idx_k = np.arange(128)[:, None]
    d = idx_c - idx_k  # c - k
    W0 = np.where(d >= 0, g32[np.clip(d, 0, L - 1)], 0.0).astype(np.float32)
    W1 = np.where(d < 0, g32[np.clip(d + 128, 0, L - 1)], 0.0).astype(np.float32)
    return W0, W1


def _build_program(mm_dt):
    """Fully static pipeline; all three DMA queue streams run in parallel.

    Input (fp16, +1 halo col per chunk): chunk sizes [1024,1024,2048,2048,
    2048] cols.  SP HWDGE carries the tiny weight loads first (their
    completion sems gate the PE's first real matmul and SP's ring drains
    FIFO), then c0,c1,c2; ACT HWDGE carries c3; GPSIMD SWDGE carries c4.
    All issued at t=0.

    PE warm-up: while waiting for c0, the PE runs 16 zero matmuls from a
    memset tile so the HAM clock-gate reaches full speed (2.4 GHz) right as
    the real stream begins (cold matmuls run at half clock).

    psum: 8 blocks of BP=1024 cols across 4 [128,1024] PSUM tensors (all 8
    banks); per block 4 matmuls (w0/w1 x 2 x N=512), each 512-col sub-tile
    reading from the single chunk buffer containing its window.  PSUM->SBUF
    drain is split by bank: DVE casts cols [0,512) f32->fp16, ACT casts
    [512,1024), in parallel, into 4 [128,1024] fp16 yout slots.

    Output (fp16; host upcasts): 8 pieces of 1024 cols (256 KiB), piece p
    issued once both half-copies land: GPSIMD takes p0,p2,p4; ACT takes
    p1,p3,p5,p6; SP (idle after its inputs) and ACT split the tail p7 in
    half, each shipping one half-copy the moment it lands.

    All waits are standalone WAIT instructions, and no DMA-completion
    threshold can be met by a mixture of partial completions from different
    transfers (per-chunk input sems; per-slot output sems whose cross-engine
    reuse is ordered by the copy/issue dependency chain)."""
    f32 = mybir.dt.float32
    BP = 1024
    NP = J // BP         # 8 psum blocks / output pieces
    CUTS = [0, 1024, 2048, 3072, 4096, 6144, J]
    NCH = len(CUTS) - 1
    N_WARMUP = 16
    # chunk->ring placement chosen so completion order matches the PE's
    # consumption order: only the lead chunks c0,c1 ride SP (short ring ->
    # earliest completion sems), c2 then c4 on ACT, c3 on GPSIMD
    IN_ENG = {0: "sp", 1: "sp", 2: "sp", 3: "act", 5: "act", 4: "pool"}
    OUT_ENG = {0: "pool", 2: "pool", 4: "pool", 1: "act", 3: "act", 5: "act", 6: "act", 7: "sp"}
    nc = bass.Bass()
    R = nc.dram_tensor("R", [128, J + 1], mm_dt, kind="ExternalInput")
    W0 = nc.dram_tensor("W0", [128, 128], mm_dt, kind="ExternalInput")
    W1 = nc.dram_tensor("W1", [128, 128], mm_dt, kind="ExternalInput")
    Y = nc.dram_tensor("Y", [128, J], mm_dt, kind="ExternalOutput")

    import contextlib

    with contextlib.ExitStack() as ctx:
        w0t = ctx.enter_context(nc.sbuf_tensor([128, 128], mm_dt))
        w1t = ctx.enter_context(nc.sbuf_tensor([128, 128], mm_dt))
        rin_all = ctx.enter_context(nc.sbuf_tensor([128, J + NCH], mm_dt))
        yout_all = ctx.enter_context(nc.sbuf_tensor([128, 8 * BP], mm_dt))
        dz = ctx.enter_context(nc.sbuf_tensor([128, 512], mm_dt))
        pts = [
            ctx.enter_context(nc.psum_tensor(f"pt{i}", [128, BP], f32))
            for i in range(4)
        ]
        s_w = ctx.enter_context(nc.semaphore("s_w"))
        s_z = ctx.enter_context(nc.semaphore("s_z"))
        s_in = [ctx.enter_context(nc.semaphore(f"s_in{i}")) for i in range(NCH)]
        s_yo = [ctx.enter_context(nc.semaphore(f"s_yo{i}")) for i in range(NP)]
        s_pe = ctx.enter_context(nc.semaphore("s_pe"))
        s_cpv = ctx.enter_context(nc.semaphore("s_cpv"))
        s_cpa = ctx.enter_context(nc.semaphore("s_cpa"))
        s_yo7a = ctx.enter_context(nc.semaphore("s_yo7a"))
        s_yo7b = ctx.enter_context(nc.semaphore("s_yo7b"))
        block = ctx.enter_context(nc.Block())

        # rin[c] holds R cols [CUTS[c], CUTS[c+1]] inclusive (left halo col)
        rin_off = [CUTS[c] + c for c in range(NCH)]
        rin = [
            rin_all[:, rin_off[c] : rin_off[c] + (CUTS[c + 1] - CUTS[c]) + 1]
            for c in range(NCH)
        ]
        yout = [yout_all[:, i * BP : (i + 1) * BP] for i in range(NP)]

        def chunk_of_sub(k):
            # 512-col sub-tile k reads buffer cols [512k, 512(k+1)] inclusive
            for c in range(NCH):
                if CUTS[c] <= 512 * k and 512 * (k + 1) <= CUTS[c + 1]:
                    return c
            raise AssertionError(k)

        def in_chunk(engine, c):
            engine.dma_start(
                out=rin[c], in_=R[:, CUTS[c] : CUTS[c + 1] + 1]
            ).then_inc(s_in[c], 16)

        def out_piece(engine, p, own_copy=False):
            engine.wait_ge(s_cpv, p + 1)
            if not own_copy:
                engine.wait_ge(s_cpa, p + 1)
            engine.dma_start(
                out=Y[:, p * BP : (p + 1) * BP], in_=yout[p]
            ).then_inc(s_yo[p], 16)

        def final_waits(engine, pieces):
            for p in sorted(pieces):
                engine.wait_ge(s_yo[p], 16)

        def eng_pieces(name):
            return sorted(p for p, e in OUT_ENG.items() if e == name)

        def eng_chunks(name):
            return sorted(c for c, e in IN_ENG.items() if e == name)

        @block.sync
        def _(sync):
            sync.dma_start(out=w0t[:, :], in_=W0[:, :]).then_inc(s_w, 16)
            sync.dma_start(out=w1t[:, :], in_=W1[:, :]).then_inc(s_w, 16)
            for c in eng_chunks("sp"):
                in_chunk(sync, c)
            # tail piece p7 is split: SP carries the DVE half only
            sync.wait_ge(s_cpv, NP)
            sync.dma_start(
                out=Y[:, (NP - 1) * BP : (NP - 1) * BP + 512],
                in_=yout[NP - 1][:, :512],
            ).then_inc(s_yo7a, 16)
            sync.wait_ge(s_yo7a, 16)

        @block.tensor
        def _(tensor):
            # HAM warm-up on zeros while the first chunk is in flight
            tensor.wait_ge(s_z, 1)
            for _ in range(N_WARMUP):
                nc.tensor.matmul(
                    pts[0][:, :512], dz[:, :128], dz[:, :], start=True, stop=True
                )
            tensor.wait_ge(s_w, 32)
            waited = set()
            for p in range(NP):
                subs = [2 * p, 2 * p + 1]
                for k in subs:
                    ch = chunk_of_sub(k)
                    if ch not in waited:
                        tensor.wait_ge(s_in[ch], 16)
                        waited.add(ch)
                if p >= 4:
                    # psum slot p%4 drained by both copy engines
                    tensor.wait_ge(s_cpv, p - 3)
                    tensor.wait_ge(s_cpa, p - 3)
                pt = pts[p % 4]
                for s, k in enumerate(subs):
                    ch = chunk_of_sub(k)
                    off = 512 * k - CUTS[ch]
                    nc.tensor.matmul(
                        pt[:, s * 512 : (s + 1) * 512],
                        w0t[:, :],
                        rin[ch][:, off + 1 : off + 513],
                        start=True,
                        stop=False,
                    )
                for s, k in enumerate(subs):
                    ch = chunk_of_sub(k)
                    off = 512 * k - CUTS[ch]
                    mm = nc.tensor.matmul(
                        pt[:, s * 512 : (s + 1) * 512],
                        w1t[:, :],
                        rin[ch][:, off : off + 512],
                        start=False,
                        stop=True,
                    )
                mm.then_inc(s_pe, 1)

        @block.vector
        def _(vector):
            nc.vector.memset(dz[:, :], 0).then_inc(s_z, 1)
            for p in range(NP):
                vector.wait_ge(s_pe, p + 1)
                nc.vector.tensor_copy(
                    out=yout[p][:, :512],
                    in_=pts[p % 4][:, :512],
                ).then_inc(s_cpv, 1)

        @block.scalar
        def _(scalar):
            for c in eng_chunks("act"):
                in_chunk(scalar, c)
            act_out = set(eng_pieces("act"))
            for p in range(NP):
                scalar.wait_ge(s_pe, p + 1)
                nc.scalar.copy(
                    out=yout[p][:, 512:],
                    in_=pts[p % 4][:, 512:],
                ).then_inc(s_cpa, 1)
                if p in act_out:
                    out_piece(scalar, p, own_copy=True)
                if p == NP - 1:
                    # tail piece is split: ACT ships its own copied half,
                    # in parallel with SP's half
                    scalar.dma_start(
                        out=Y[:, p * BP + 512 : (p + 1) * BP],
                        in_=yout[p][:, 512:],
                    ).then_inc(s_yo7b, 16)
            final_waits(scalar, eng_pieces("act"))
            scalar.wait_ge(s_yo7b, 16)

        @block.gpsimd
        def _(gpsimd):
            for c in eng_chunks("pool"):
                in_chunk(gpsimd, c)
            for p in eng_pieces("pool"):
                out_piece(gpsimd, p)
            final_waits(gpsimd, eng_pieces("pool"))

    return nc


def _get_program(mm_dt):
    key = str(mm_dt)
    if key not in _PROGRAM_CACHE:
        _PROGRAM_CACHE[key] = _build_program(mm_dt)
    return _PROGRAM_CACHE[key]


def _device_run(in_maps, _trace: bool = False, **_trace_kwargs):
    """Execute the SPMD program; returns the per-core Y arrays."""
    global LAST_RESULTS
    nc = _get_program(MM_DT)
    res = run_bass_kernel_spmd(
        nc, in_maps, core_ids=list(range(N_CORES)), trace=_trace, **_trace_kwargs
    )
    LAST_RESULTS = res
    return [res.results[m]["Y"] for m in range(N_CORES)]


def _device_run_subprocess(in_maps):
    """Run the kernel in a fresh python process.

    The axon-tunneled runtime occasionally reports a transient
    NRT_EXEC_UNIT_UNRECOVERABLE on an execute; the wedged state does not
    clear within the process, but a fresh process (fresh NRT session)
    recovers reliably."""
    import os
    import subprocess
    import sys
    import tempfile

    tmp = tempfile.mkdtemp()
    in_path = os.path.join(tmp, "in.npz")
    out_path = os.path.join(tmp, "out.npz")
    flat = {}
    for m, im in enumerate(in_maps):
        for k, v in im.items():
            flat[f"{k}_{m}"] = v
    np.savez(in_path, **flat)
    subprocess.run(
        [sys.executable, os.path.abspath(__file__), "--device-run", in_path, out_path],
        check=True,
    )
    with np.load(out_path) as z:
        return [z[f"Y_{m}"] for m in range(N_CORES)]


def kernel(r, b, a, _trace: bool = False, **_trace_kwargs):
    r = np.ascontiguousarray(np.asarray(r, dtype=np.float32))
    assert r.shape == (T,), r.shape

    g = _fir_coeffs(b, a)
    W0, W1 = _weight_mats(g)

    np_dt = mybir.dt.np(MM_DT)  # float16 for 2-byte operands, else float32
    W0 = W0.astype(np_dt)
    W1 = W1.astype(np_dt)

    in_maps = []
    for m in range(N_CORES):
        halo = (
            np.zeros(128, np.float32) if m == 0 else r[m * C - 128 : m * C]
        )
        rbuf = np.concatenate([halo, r[m * C : (m + 1) * C]]).astype(np_dt)
        R = np.ascontiguousarray(rbuf.reshape(J + 1, 128).T)
        in_maps.append({"R": R, "W0": W0, "W1": W1})

    try:
        ys = _device_run(in_maps, _trace=_trace, **_trace_kwargs)
    except Exception:
        ys = _device_run_subprocess(in_maps)

    y = np.concatenate(
        [
            np.ascontiguousarray(ys[m].astype(np.float32, copy=False).T).reshape(-1)
            for m in range(N_CORES)
        ]
    )
    return y


if __name__ == "__main__":
    import sys as _sys

    if len(_sys.argv) == 4 and _sys.argv[1] == "--device-run":
        with np.load(_sys.argv[2]) as _z:
            _in_maps = [
                {k: _z[f"{k}_{m}"] for k in ("R", "W0", "W1")}
                for m in range(N_CORES)
            ]
        _ys = _device_run(_in_maps)
        np.savez(_sys.argv[3], **{f"Y_{m}": _ys[m] for m in range(N_CORES)})
